# revision 1
# baseline (speedup 1.0000x reference)
"""Causal self-attention (B=4, T=2048, D=1024, H=16) on 8 TRN2 NeuronCores.

Sharding: core c handles batch b = c//2 and head-group g = c%2 (8 heads each).
Each core computes, for its (b, g):
    qkv_loc = x[b] @ w_qkv[:, cols(g)]          (q|k|v local, 512 cols each)
    att     = causal_attention(q, k, v)          (8 heads, hd=64)
    y_part  = att @ w_out[rows(g), :]            ([2048, 1024] partial)
Host sums the two partial outputs per batch.

TensorEngine matmuls run in MM dtype (bf16 / f32 / f32r) with fp32 PSUM
accumulation. Softmax uses exp on ScalarE with deferred normalization:
rowsums come free from a ones-column appended to V, and the reciprocal is
broadcast across partitions with a K=1 outer-product matmul.
"""

import os

import numpy as np

import concourse.bass as bass
import concourse.mybir as mybir
from concourse import bacc, tile
from concourse import bass_utils
from concourse.masks import make_identity

# Problem constants (hardcoded per contest contract)
B = 4
T = 2048
D = 1024
H = 16
HD = 64
H_LOC = 8               # heads per core
CLOC = H_LOC * HD       # 512 local head dims
P = 128
N_CORES = 8

F32 = mybir.dt.float32
F32R = mybir.dt.float32r
BF16 = mybir.dt.bfloat16

# Compute dtype knob: "bf16" | "f32" | "f32r"
MM_MODE = os.environ.get("ATTN_MM_MODE", "f32r")
_MM_MAP = {"bf16": BF16, "f32": F32, "f32r": F32R}


def _build_kernel_body(nc, tc, x_ap, wqkv_ap, wout_ap, out_ap, mm):
    from contextlib import ExitStack

    Exp = mybir.ActivationFunctionType.Exp
    mult = mybir.AluOpType.mult

    is_bf16 = mm == BF16
    is_f32r = mm == F32R

    def bitin(ap):
        # DRAM views for direct loads into f32r tiles (bit-identical)
        return ap.bitcast(F32R) if is_f32r else ap

    ctx = ExitStack()
    # ---------------- constants ----------------
    const = ctx.enter_context(tc.tile_pool(name="const", bufs=1))
    ident = const.tile([P, P], mm)
    if is_f32r:
        # gpsimd memset/affine_select can't write f32r; build f32, copy-cast
        scratch = const.tile([P, P], F32, tag="idscratch")
        make_identity(nc, scratch)
        nc.vector.tensor_copy(ident, scratch)
        ones_f = const.tile([P, 16 * H_LOC], F32, tag="ones_f")
        nc.gpsimd.memset(ones_f, 1.0)
    else:
        make_identity(nc, ident)

    # causal mask helper: wm[p, x] = 1.0 iff p <= x - 384 else 0.0
    # (consumed only by DVE multiplies, so f32 is fine in f32r mode)
    wm_dt = BF16 if is_bf16 else F32
    wm = const.tile([P, 896], wm_dt)
    nc.gpsimd.memset(wm, 1.0)
    nc.gpsimd.affine_select(
        out=wm,
        in_=wm,
        compare_op=mybir.AluOpType.is_ge,  # keep where f - p - 384 >= 0
        fill=0.0,
        base=-384,
        channel_multiplier=-1,
        pattern=[[1, 896]],
    )

    oc = const.tile([1, 64], mm)  # ones column for rowsum broadcast
    if is_f32r:
        nc.vector.tensor_copy(oc, ones_f[0:1, 0:64])
    else:
        nc.gpsimd.memset(oc, 1.0)

    qkt_pool = ctx.enter_context(tc.tile_pool(name="qkt", bufs=1))
    QT = qkt_pool.tile([P, 4, T], mm)   # head h -> rows (h%2)*64.., subtile h//2
    KT = qkt_pool.tile([P, 4, T], mm)
    V_aug = qkt_pool.tile([P, 16, H_LOC, HD + 1], mm)  # [j%128, jb, h, dd|ones]
    if is_f32r:
        nc.vector.tensor_copy(
            V_aug[:, :, :, HD],
            ones_f.rearrange("p (a b) -> p a b", a=16),
        )
    else:
        nc.gpsimd.memset(V_aug[:, :, :, HD], 1.0)

    xa = x_ap.rearrange("(tb p) d -> tb p d", p=P)  # [16, 128, 1024]
    wqk = wqkv_ap[:, 0:2 * CLOC].rearrange("(o p) c -> p o c", p=P)
    wv = wqkv_ap[:, 2 * CLOC:3 * CLOC].rearrange("(o p) c -> p o c", p=P)

    ch = CLOC // 2

    def load_cast(pool, shape, tag, src):
        """DMA an f32 DRAM region into an mm-dtype tile."""
        if is_bf16:
            st = pool.tile(shape, F32, tag=tag + "_st")
            nc.sync.dma_start(st, src)
            t = pool.tile(shape, mm, tag=tag)
            nc.vector.tensor_copy(t, st)
            return t
        t = pool.tile(shape, mm, tag=tag)
        nc.sync.dma_start(t, bitin(src))
        return t

    with tc.tile_pool(name="xt", bufs=1) as xt_pool, \
         tc.tile_pool(name="psAB", bufs=2, space="PSUM") as psum:
        xT = xt_pool.tile([P, 8, T], mm)  # [d%128, d//128, t]

        # ---- phase A: x -> xT (transpose+cast), V-proj c-half 0 fused ----
        with tc.tile_pool(name="lda", bufs=2) as lda, \
             tc.tile_pool(name="ldv", bufs=1) as ldv:
            wv_sb = load_cast(ldv, [P, 8, ch], "wv", wv[:, :, 0:ch])
            for tb in range(T // P):
                if is_bf16:
                    xin = lda.tile([P, D], F32, tag="xin")
                    nc.sync.dma_start(xin, xa[tb])
                    xc = lda.tile([P, D], mm, tag="xc")
                    nc.vector.tensor_copy(xc, xin)
                else:
                    xc = lda.tile([P, D], mm, tag="xin")
                    nc.sync.dma_start(xc, bitin(xa[tb]))
                for db in range(D // P):
                    pt = psum.tile([P, P], mm, tag="ps_t")
                    nc.tensor.transpose(pt, xc[:, db * P:(db + 1) * P], ident)
                    nc.vector.tensor_copy(xT[:, db, tb * P:(tb + 1) * P], pt)
                ps = psum.tile([P, ch], F32, tag="ps_v")
                for k in range(8):
                    nc.tensor.matmul(
                        ps,
                        xT[:, k, tb * P:(tb + 1) * P],
                        wv_sb[:, k, :],
                        start=(k == 0),
                        stop=(k == 7),
                    )
                nc.vector.tensor_copy(
                    V_aug[:, tb, 0:4, 0:HD],
                    ps.rearrange("p (h d) -> p h d", h=H_LOC // 2),
                )

        # ---- V-proj c-half 1 ----
        with tc.tile_pool(name="ldv2", bufs=1) as ldv2:
            wv_sb = load_cast(ldv2, [P, 8, ch], "wv2", wv[:, :, ch:2 * ch])
            for tb in range(T // P):
                ps = psum.tile([P, ch], F32, tag="ps_v")
                for k in range(8):
                    nc.tensor.matmul(
                        ps,
                        xT[:, k, tb * P:(tb + 1) * P],
                        wv_sb[:, k, :],
                        start=(k == 0),
                        stop=(k == 7),
                    )
                nc.vector.tensor_copy(
                    V_aug[:, tb, 4:8, 0:HD],
                    ps.rearrange("p (h d) -> p h d", h=H_LOC // 2),
                )

        # ---- phase B: Q^T / K^T proj: psum[c_block 128, t 512] ----
        with tc.tile_pool(name="ldw", bufs=2) as ldw:
            for cb in range(8):
                wcb = load_cast(
                    ldw, [P, 8, P], "wst", wqk[:, :, cb * P:(cb + 1) * P]
                )
                dest = QT if cb < 4 else KT
                sub = cb % 4
                for it in range(4):
                    ps = psum.tile([P, 512], F32, tag="ps_qkv")
                    for k in range(8):
                        nc.tensor.matmul(
                            ps,
                            wcb[:, k, :],
                            xT[:, k, it * 512:(it + 1) * 512],
                            start=(k == 0),
                            stop=(k == 7),
                        )
                    nc.vector.tensor_copy(dest[:, sub, it * 512:(it + 1) * 512], ps)

    # ---------------- phase C: causal attention ----------------
    # Scores matmuls must contract over K=128 partitions: K<96 never warms
    # the PE HAM clock gate (stuck at 1.2 GHz). KT is packed (2 heads = 128
    # real rows) as lhsT; the moving Q operand is a per-parity scratch with
    # the *other* head's 64 rows zeroed, so the packed-KT contraction picks
    # out exactly one head at full K=128.
    atp = ctx.enter_context(tc.tile_pool(name="atp", bufs=1))
    AT = atp.tile([P, 4, T], mm)        # attention output, laid out like QT
    Qp0 = atp.tile([P, T], mm)          # padded Q scratch, even heads
    Qp1 = atp.tile([P, T], mm)          # padded Q scratch, odd heads
    with tc.tile_pool(name="att", bufs=3) as att_pool, \
         tc.tile_pool(name="attsm", bufs=2) as sm_pool, \
         tc.tile_pool(name="psC", bufs=2, space="PSUM") as psum:
        # zero the never-written halves once (x*0 keeps f32r rounding legal)
        nc.vector.tensor_scalar_mul(Qp0[64:128, :], QT[64:128, 0, :], 0.0)
        nc.vector.tensor_scalar_mul(Qp1[0:64, :], QT[0:64, 0, :], 0.0)
        for h in range(H_LOC):
            row0 = (h % 2) * 64
            sub = h // 2
            Qph = Qp0 if h % 2 == 0 else Qp1
            nc.vector.tensor_copy(
                Qph[row0:row0 + 64, :], QT[row0:row0 + 64, sub, :]
            )
            for it in range(4):
                i0 = it * 512
                njb = 4 * (it + 1)
                po = psum.tile([P, 512], F32, tag="ps_o")
                for jb2 in range(njb // 2):
                    ps = psum.tile([P, 1024], F32, tag="ps_s")
                    for u in range(2):
                        jb = 2 * jb2 + u
                        nc.tensor.matmul(
                            ps[:, u * 512:(u + 1) * 512],
                            KT[:, sub, jb * P:(jb + 1) * P],
                            Qph[:, i0:i0 + 512],
                            start=True,
                            stop=True,
                        )
                    es = att_pool.tile([P, 1024], mm, tag="es")
                    nc.scalar.activation(es, ps, Exp, scale=0.125)
                    for u in range(2):
                        jb = 2 * jb2 + u
                        off = jb * P - i0
                        if off >= 0:  # diagonal region: zero out j > i
                            s = 384 - off
                            nc.vector.tensor_tensor(
                                es[:, u * 512:(u + 1) * 512],
                                es[:, u * 512:(u + 1) * 512],
                                wm[:, s:s + 512],
                                mult,
                            )
                    for u in range(2):
                        jb = 2 * jb2 + u
                        nc.tensor.matmul(
                            po[0:HD + 1, :],
                            V_aug[:, jb, h, :],
                            es[:, u * 512:(u + 1) * 512],
                            start=(jb == 0),
                            stop=(jb == njb - 1),
                        )
                # deferred softmax normalization
                rr = sm_pool.tile([1, 512], F32, tag="rr")
                nc.vector.tensor_copy(rr, po[HD:HD + 1, :])
                nc.vector.reciprocal(rr, rr)
                if mm != F32:
                    rm = sm_pool.tile([1, 512], mm, tag="rm")
                    nc.vector.tensor_copy(rm, rr)
                else:
                    rm = rr
                pb = psum.tile([64, 512], F32, tag="ps_b")
                nc.tensor.matmul(pb, oc, rm, start=True, stop=True)
                rb = sm_pool.tile([64, 512], F32, tag="rb")
                nc.vector.tensor_copy(rb, pb)
                nc.vector.tensor_tensor(
                    AT[row0:row0 + 64, sub, i0:i0 + 512],
                    po[0:64, :],
                    rb,
                    mult,
                )

    # ---------------- phase D: output projection ----------------
    wo = wout_ap.rearrange("(o p) n -> p o n", p=P)  # [128, 4, 1024]
    oa = out_ap.rearrange("(tb p) d -> tb p d", p=P)
    with tc.tile_pool(name="ldo", bufs=2) as ldo, \
         tc.tile_pool(name="ypool", bufs=3) as ypool, \
         tc.tile_pool(name="psD", bufs=4, space="PSUM") as psum:
        if is_bf16:
            wo_st = ldo.tile([P, 4, D], F32, tag="wo_st")
            nc.sync.dma_start(wo_st, wo)
            wo_sb = ldo.tile([P, 4, D], mm, tag="wo_sb")
            nc.vector.tensor_copy(wo_sb, wo_st)
        else:
            wo_sb = ldo.tile([P, 4, D], mm, tag="wo_st")
            nc.sync.dma_start(wo_sb, bitin(wo))
        for tb in range(T // P):
            for nt in range(2):
                py = psum.tile([P, 512], F32, tag="ps_y")
                for k in range(4):
                    nc.tensor.matmul(
                        py,
                        AT[:, k, tb * P:(tb + 1) * P],
                        wo_sb[:, k, nt * 512:(nt + 1) * 512],
                        start=(k == 0),
                        stop=(k == 3),
                    )
                ysb = ypool.tile([P, 512], F32, tag="ysb")
                nc.vector.tensor_copy(ysb, py)
                nc.sync.dma_start(oa[tb, :, nt * 512:(nt + 1) * 512], ysb)

    ctx.close()


_CACHE = {}


def _get_nc(mode=None):
    mode = mode or MM_MODE
    if mode in _CACHE:
        return _CACHE[mode]
    mm = _MM_MAP[mode]
    nc = bacc.Bacc(
        "TRN2",
        target_bir_lowering=False,
        debug=False,
        enable_asserts=False,
        num_devices=N_CORES,
    )
    x_d = nc.dram_tensor("x", [T, D], F32, kind="ExternalInput")
    wqkv_d = nc.dram_tensor("w_qkv", [D, 3 * CLOC], F32, kind="ExternalInput")
    wout_d = nc.dram_tensor("w_out", [CLOC, D], F32, kind="ExternalInput")
    out_d = nc.dram_tensor("out", [T, D], F32, kind="ExternalOutput")
    with tile.TileContext(nc) as tc:
        _build_kernel_body(
            nc, tc, x_d.ap(), wqkv_d.ap(), wout_d.ap(), out_d.ap(), mm
        )
    nc.compile()
    _CACHE[mode] = nc
    return nc


def _make_in_maps(x, w_qkv, w_out):
    x = np.ascontiguousarray(np.asarray(x, dtype=np.float32))
    w_qkv = np.ascontiguousarray(np.asarray(w_qkv, dtype=np.float32))
    w_out = np.ascontiguousarray(np.asarray(w_out, dtype=np.float32))
    in_maps = []
    for c in range(N_CORES):
        b, g = divmod(c, 2)
        c0 = g * CLOC
        wloc = np.concatenate(
            [
                w_qkv[:, c0:c0 + CLOC],
                w_qkv[:, D + c0:D + c0 + CLOC],
                w_qkv[:, 2 * D + c0:2 * D + c0 + CLOC],
            ],
            axis=1,
        )
        in_maps.append({
            "x": np.ascontiguousarray(x[b]),
            "w_qkv": np.ascontiguousarray(wloc),
            "w_out": np.ascontiguousarray(w_out[c0:c0 + CLOC]),
        })
    return in_maps


def run(x, w_qkv, w_out, trace=False, mode=None):
    nc = _get_nc(mode)
    in_maps = _make_in_maps(x, w_qkv, w_out)
    res = bass_utils.run_bass_kernel_spmd(
        nc, in_maps, core_ids=list(range(N_CORES)), trace=trace
    )
    y = np.empty((B, T, D), dtype=np.float32)
    for b in range(B):
        y[b] = res.results[2 * b]["out"] + res.results[2 * b + 1]["out"]
    return y, res


def kernel(x, w_qkv, w_out):
    y, _ = run(x, w_qkv, w_out, trace=False)
    return y



# revision 7
# speedup vs baseline: 1.4910x; 1.4910x over previous
"""Causal self-attention (B=4, T=2048, D=1024, H=16) on 8 TRN2 NeuronCores.

Sharding: core c handles batch b = c//2 and head-group g = c%2 (8 heads each).
Each core computes, for its (b, g):
    qkv_loc = x[b] @ w_qkv[:, cols(g)]          (q|k|v local, 512 cols each)
    att     = causal_attention(q, k, v)          (8 heads, hd=64)
    y_part  = att @ w_out[rows(g), :]            ([2048, 1024] partial)
Host sums the two partial outputs per batch.

All matmuls run in bf16 with fp32 PSUM accumulation. Softmax uses exp on
ScalarE with deferred normalization: rowsums come free from a ones-column
appended to V, the reciprocal is a single-pass Newton-Raphson approximation
read straight out of PSUM, and the result is broadcast across partitions
with a K=1 outer-product matmul.

Phase structure (single emission stream; engines overlap via Tile deps):
  A  x -> xT (cast on ScalarE + PE transpose), V projection fused in
  B0 Q/K projection for head-pair 0
  C  per head-pair `sub`: attention; Q/K projection for pair sub+1 is
     interleaved one matmul-group per (head, window) block so the PE stays
     fed while ScalarE runs exp; pair 3 interleaves the output projection
     instead.  Softmax normalization for block i is emitted during block
     i+1 (one-deep software pipeline) so its DVE chain never stalls the PE.
Causal masking: key-blocks fully above the diagonal are skipped; the
scores matmul / attention-V matmul are narrowed to the live band and only
the 128x128 diagonal triangle gets a mask multiply.
"""

import numpy as np
from contextlib import ExitStack

import concourse.bass as bass
import concourse.mybir as mybir
from concourse import bacc, tile
from concourse import bass_utils
from concourse.masks import make_identity

# Problem constants (hardcoded per contest contract)
B = 4
T = 2048
D = 1024
H = 16
HD = 64
H_LOC = 8               # heads per core
CLOC = H_LOC * HD       # 512 local head dims
P = 128
N_CORES = 8

F32 = mybir.dt.float32
BF16 = mybir.dt.bfloat16
MM_MODE = "bf16"


def _build_kernel_body(nc, tc, x_ap, wqkv_ap, wout_ap, out_ap):
    Exp = mybir.ActivationFunctionType.Exp
    mult = mybir.AluOpType.mult

    ctx = ExitStack()

    # ---------------- constants ----------------
    const = ctx.enter_context(tc.tile_pool(name="const", bufs=1))
    ident = const.tile([P, P], BF16)
    make_identity(nc, ident)
    # causal keep-mask for a 128x128 diagonal block: wm[k, q] = 1.0 iff q >= k
    wm = const.tile([P, P], BF16)
    nc.gpsimd.memset(wm, 1.0)
    nc.gpsimd.affine_select(
        out=wm,
        in_=wm,
        compare_op=mybir.AluOpType.is_ge,  # keep where f - p >= 0
        fill=0.0,
        base=0,
        channel_multiplier=-1,
        pattern=[[1, P]],
    )
    oc = const.tile([1, HD], BF16)  # ones row for rowsum-reciprocal broadcast
    nc.gpsimd.memset(oc, 1.0)

    big = ctx.enter_context(tc.tile_pool(name="big", bufs=1))
    xT = big.tile([P, 8, T], BF16)      # [d%128, d//128, t]
    QT = big.tile([P, 4, T], BF16)      # head h -> rows (h%2)*64.., subtile h//2
    KT = big.tile([P, 4, T], BF16)
    V_aug = big.tile([P, 16, H_LOC, HD + 1], BF16)  # [j%128, jb, h, dd|ones]
    AT = big.tile([P, 4, T], BF16)      # attention output, laid out like QT
    Qp0 = big.tile([P, T], BF16)        # padded Q scratch, even heads
    Qp1 = big.tile([P, T], BF16)        # padded Q scratch, odd heads
    nc.gpsimd.memset(V_aug[:, :, :, HD], 1.0)
    nc.gpsimd.memset(Qp0[64:128, :], 0.0)
    nc.gpsimd.memset(Qp1[0:64, :], 0.0)

    xa = x_ap.rearrange("(tb p) d -> tb p d", p=P)          # [16, 128, 1024]
    wqk = wqkv_ap[:, 0:2 * CLOC].rearrange("(o p) c -> p o c", p=P)
    wv = wqkv_ap[:, 2 * CLOC:3 * CLOC].rearrange("(o p) c -> p o c", p=P)
    wo = wout_ap.rearrange("(o p) n -> p o n", p=P)         # [128, 4, 1024]
    oa = out_ap.rearrange("(tb p) d -> tb p d", p=P)

    wpre = ctx.enter_context(tc.tile_pool(name="wpre", bufs=1))
    wv_sb = wpre.tile([P, 8, CLOC], BF16)
    wo_sb = wpre.tile([P, 4, D], BF16)

    # ---- phase A: x -> xT (cast+transpose), V projection fused ----
    with tc.tile_pool(name="stage", bufs=1) as stage, \
         tc.tile_pool(name="lda", bufs=3) as lda, \
         tc.tile_pool(name="psA", bufs=4, space="PSUM") as psA, \
         tc.tile_pool(name="psV", bufs=2, space="PSUM") as psV:

        def load_x(tb):
            xin = lda.tile([P, D], F32, tag="xin")
            nc.sync.dma_start(xin, xa[tb])
            xc = lda.tile([P, D], BF16, tag="xc")
            nc.scalar.copy(xc, xin)  # cast on ScalarE; DVE is busier here
            return xc

        def vproj(tb):
            ps = psV.tile([P, CLOC], F32, tag="ps_v")
            for k in range(8):
                nc.tensor.matmul(
                    ps,
                    xT[:, k, tb * P:(tb + 1) * P],
                    wv_sb[:, k, :],
                    start=(k == 0),
                    stop=(k == 7),
                )
            nc.vector.tensor_copy(
                V_aug[:, tb, :, 0:HD],
                ps.rearrange("p (h d) -> p h d", h=H_LOC),
            )

        xc_cur = load_x(0)
        # weight loads after the first x tile so they don't delay phase A
        wv_st = stage.tile([P, 8, CLOC], F32, tag="wv_st")
        nc.sync.dma_start(wv_st, wv)
        nc.vector.tensor_copy(wv_sb, wv_st)
        wo_st = stage.tile([P, 4, D], F32, tag="wo_st")
        nc.sync.dma_start(wo_st, wo)

        for tb in range(T // P):
            xc_next = load_x(tb + 1) if tb + 1 < T // P else None
            # one psum tile per transpose: a matmul with start=True clears
            # the whole destination bank, so slices of one bank can't be
            # written by separate transposes
            for db in range(8):
                pt = psA.tile([P, P], BF16, tag="pt")
                nc.tensor.transpose(pt, xc_cur[:, db * P:(db + 1) * P], ident)
                nc.vector.tensor_copy(xT[:, db, tb * P:(tb + 1) * P], pt)
            if tb > 0:
                vproj(tb - 1)  # one-deep pipeline behind the transposes
            xc_cur = xc_next
        vproj(T // P - 1)
        nc.vector.tensor_copy(wo_sb, wo_st)

    # ---- phase B/C/D machinery ----
    ldw = ctx.enter_context(tc.tile_pool(name="ldw", bufs=2))
    psB = ctx.enter_context(tc.tile_pool(name="psB", bufs=1, space="PSUM"))

    def b_load_pair(pair):
        tiles = []
        for cb in (pair, pair + 4):
            wst = ldw.tile([P, 8, P], F32, tag="wst")
            nc.sync.dma_start(wst, wqk[:, :, cb * P:(cb + 1) * P])
            wcb = ldw.tile([P, 8, P], BF16, tag="wcb")
            nc.vector.tensor_copy(wcb, wst)
            tiles.append(wcb)
        return tiles

    def b_group(pair, tiles, gi):
        ci, it = divmod(gi, 4)
        cb = pair + 4 * ci
        dest = QT if cb < 4 else KT
        sub = cb % 4
        ps = psB.tile([P, 512], F32, tag="psb")
        for k in range(8):
            nc.tensor.matmul(
                ps,
                tiles[ci][:, k, :],
                xT[:, k, it * 512:(it + 1) * 512],
                start=(k == 0),
                stop=(k == 7),
            )
        nc.vector.tensor_copy(dest[:, sub, it * 512:(it + 1) * 512], ps)

    # Q/K projection for head-pair 0 runs up front
    pair0 = b_load_pair(0)
    for gi in range(8):
        b_group(0, pair0, gi)

    # ---- phase C: causal attention ----
    # Scores matmuls contract over K=128 partitions (K<96 never warms the
    # PE HAM clock gate).  KT is packed (2 heads = 128 rows) as lhsT; the
    # moving Q operand is a per-parity scratch with the other head's 64
    # rows zeroed, so the packed-KT contraction picks out exactly one head.
    attp = ctx.enter_context(tc.tile_pool(name="att", bufs=3))
    smp = ctx.enter_context(tc.tile_pool(name="sm", bufs=2))
    ypool = ctx.enter_context(tc.tile_pool(name="ypool", bufs=3))
    psS = ctx.enter_context(tc.tile_pool(name="psS", bufs=2, space="PSUM"))
    psO = ctx.enter_context(tc.tile_pool(name="psO", bufs=2, space="PSUM"))
    psR = ctx.enter_context(tc.tile_pool(name="psR", bufs=1, space="PSUM"))

    def norm(pend):
        # softmax normalization for a finished (head, window) block
        po, row0, sub, i0 = pend
        rs = smp.tile([1, 512], F32, tag="rs")
        # copy to partition 0 first: reciprocal_approx_fast (custom DVE op)
        # mishandles a nonzero input partition offset
        nc.vector.tensor_copy(rs, po[HD:HD + 1, :])
        rr = smp.tile([1, 512], F32, tag="rr")
        nc.vector.reciprocal_approx_fast(rr, rs)
        rm = smp.tile([1, 512], BF16, tag="rm")
        nc.vector.tensor_copy(rm, rr)
        pb = psR.tile([HD, 512], F32, tag="pb")
        nc.tensor.matmul(pb, oc, rm, start=True, stop=True)
        rb = smp.tile([HD, 512], BF16, tag="rb")
        nc.vector.tensor_copy(rb, pb)
        nc.vector.tensor_tensor(
            AT[row0:row0 + 64, sub, i0:i0 + 512], po[0:HD, :], rb, mult
        )

    def d_group(it):
        # output projection for query blocks [4*it, 4*it+4)
        for tb in range(4 * it, 4 * it + 4):
            for nt in range(2):
                py = psB.tile([P, 512], F32, tag="psb")
                for k in range(4):
                    nc.tensor.matmul(
                        py,
                        AT[:, k, tb * P:(tb + 1) * P],
                        wo_sb[:, k, nt * 512:(nt + 1) * 512],
                        start=(k == 0),
                        stop=(k == 3),
                    )
                ysb = ypool.tile([P, 512], F32, tag="ysb")
                nc.vector.tensor_copy(ysb, py)
                nc.sync.dma_start(oa[tb, :, nt * 512:(nt + 1) * 512], ysb)

    pending = None
    for sub in range(4):
        ntiles = b_load_pair(sub + 1) if sub < 3 else None
        slot = 0
        for parity in (0, 1):
            h = 2 * sub + parity
            row0 = parity * 64
            Qph = Qp0 if parity == 0 else Qp1
            nc.vector.tensor_copy(
                Qph[row0:row0 + 64, :], QT[row0:row0 + 64, sub, :]
            )
            for it in range(4):
                i0 = it * 512
                njb = 4 * (it + 1)
                po = psO.tile([P, 512], F32, tag="po")
                for jb2 in range(njb // 2):
                    ps = psS.tile([P, 1024], F32, tag="ps_s")
                    es = attp.tile([P, 1024], BF16, tag="es")
                    for u in (0, 1):
                        jb = 2 * jb2 + u
                        off = max(jb * P - i0, 0)
                        nc.tensor.matmul(
                            ps[:, u * 512 + off:(u + 1) * 512],
                            KT[:, sub, jb * P:(jb + 1) * P],
                            Qph[:, i0 + off:i0 + 512],
                            start=True,
                            stop=True,
                        )
                    # exp also covers stale cols left of the diagonal; they
                    # are never read downstream
                    nc.scalar.activation(es, ps, Exp, scale=0.125)
                    for u in (0, 1):
                        jb = 2 * jb2 + u
                        off = jb * P - i0
                        if off >= 0:  # 128x128 diagonal triangle
                            nc.vector.tensor_tensor(
                                es[:, u * 512 + off:u * 512 + off + P],
                                es[:, u * 512 + off:u * 512 + off + P],
                                wm,
                                mult,
                            )
                    for u in (0, 1):
                        jb = 2 * jb2 + u
                        off = max(jb * P - i0, 0)
                        nc.tensor.matmul(
                            po[0:HD + 1, off:512],
                            V_aug[:, jb, h, :],
                            es[:, u * 512 + off:(u + 1) * 512],
                            start=(jb == 0),
                            stop=(jb == njb - 1),
                        )
                if pending is not None:
                    norm(pending)
                    if sub == 3 and parity == 1 and it >= 1:
                        d_group(it - 1)  # out-proj behind head 7's blocks
                pending = (po, row0, sub, i0)
                if ntiles is not None and slot < 8:
                    b_group(sub + 1, ntiles, slot)
                    slot += 1
    norm(pending)
    d_group(3)

    ctx.close()


_CACHE = {}


def _get_nc(mode=None):
    if "nc" in _CACHE:
        return _CACHE["nc"]
    nc = bacc.Bacc(
        "TRN2",
        target_bir_lowering=False,
        debug=False,
        enable_asserts=False,
        num_devices=N_CORES,
    )
    x_d = nc.dram_tensor("x", [T, D], F32, kind="ExternalInput")
    wqkv_d = nc.dram_tensor("w_qkv", [D, 3 * CLOC], F32, kind="ExternalInput")
    wout_d = nc.dram_tensor("w_out", [CLOC, D], F32, kind="ExternalInput")
    out_d = nc.dram_tensor("out", [T, D], F32, kind="ExternalOutput")
    with tile.TileContext(nc) as tc:
        _build_kernel_body(
            nc, tc, x_d.ap(), wqkv_d.ap(), wout_d.ap(), out_d.ap()
        )
    nc.compile()
    _CACHE["nc"] = nc
    return nc


def _make_in_maps(x, w_qkv, w_out):
    x = np.ascontiguousarray(np.asarray(x, dtype=np.float32))
    w_qkv = np.ascontiguousarray(np.asarray(w_qkv, dtype=np.float32))
    w_out = np.ascontiguousarray(np.asarray(w_out, dtype=np.float32))
    in_maps = []
    for c in range(N_CORES):
        b, g = divmod(c, 2)
        c0 = g * CLOC
        wloc = np.concatenate(
            [
                w_qkv[:, c0:c0 + CLOC],
                w_qkv[:, D + c0:D + c0 + CLOC],
                w_qkv[:, 2 * D + c0:2 * D + c0 + CLOC],
            ],
            axis=1,
        )
        in_maps.append({
            "x": np.ascontiguousarray(x[b]),
            "w_qkv": np.ascontiguousarray(wloc),
            "w_out": np.ascontiguousarray(w_out[c0:c0 + CLOC]),
        })
    return in_maps


def run(x, w_qkv, w_out, trace=False, mode=None):
    nc = _get_nc(mode)
    in_maps = _make_in_maps(x, w_qkv, w_out)
    res = bass_utils.run_bass_kernel_spmd(
        nc, in_maps, core_ids=list(range(N_CORES)), trace=trace
    )
    y = np.empty((B, T, D), dtype=np.float32)
    for b in range(B):
        y[b] = res.results[2 * b]["out"] + res.results[2 * b + 1]["out"]
    return y, res


def kernel(x, w_qkv, w_out):
    y, _ = run(x, w_qkv, w_out, trace=False)
    return y


# revision 13
# speedup vs baseline: 1.6012x; 1.0739x over previous
"""Causal self-attention (B=4, T=2048, D=1024, H=16) on 8 TRN2 NeuronCores.

Sharding: core c handles batch b = c//2 and head-group g = c%2 (8 heads each).
Each core computes, for its (b, g):
    qkv_loc = x[b] @ w_qkv[:, cols(g)]          (q|k|v local, 512 cols each)
    att     = causal_attention(q, k, v)          (8 heads, hd=64)
    y_part  = att @ w_out[rows(g), :]            ([2048, 1024] partial)
Host sums the two partial outputs per batch.

All matmuls run in bf16 with fp32 PSUM accumulation. Softmax uses exp on
ScalarE with deferred normalization: rowsums come free from a ones-column
appended to V, the reciprocal is a single-pass Newton-Raphson approximation
read straight out of PSUM, and the result is broadcast across partitions
with a K=1 outer-product matmul.

Phase structure (single emission stream; engines overlap via Tile deps):
  A  x -> xT (cast on ScalarE + PE transpose), V projection fused in
  B0 Q/K projection for head-pair 0
  C  per head-pair `sub`: attention; Q/K projection for pair sub+1 is
     interleaved one matmul-group per (head, window) block so the PE stays
     fed while ScalarE runs exp; pair 3 interleaves the output projection
     instead.  Softmax normalization for block i is emitted during block
     i+1 (one-deep software pipeline) so its DVE chain never stalls the PE.
Causal masking: key-blocks fully above the diagonal are skipped; the
scores matmul / attention-V matmul are narrowed to the live band and only
the 128x128 diagonal triangle gets a mask multiply.
"""

import numpy as np
from collections import deque
from contextlib import ExitStack

import concourse.bass as bass
import concourse.mybir as mybir
from concourse import bacc, tile
from concourse import bass_utils
from concourse.masks import make_identity

# Problem constants (hardcoded per contest contract)
B = 4
T = 2048
D = 1024
H = 16
HD = 64
H_LOC = 8               # heads per core
CLOC = H_LOC * HD       # 512 local head dims
P = 128
N_CORES = 8

F32 = mybir.dt.float32
BF16 = mybir.dt.bfloat16
MM_MODE = "bf16"


def _build_kernel_body(nc, tc, x_ap, wqkv_ap, wout_ap, out_ap):
    Exp = mybir.ActivationFunctionType.Exp
    mult = mybir.AluOpType.mult

    ctx = ExitStack()

    # ---------------- constants ----------------
    const = ctx.enter_context(tc.tile_pool(name="const", bufs=1))
    ident = const.tile([P, P], BF16)
    make_identity(nc, ident)
    # causal keep-mask for a 128x128 diagonal block: wm[k, q] = 1.0 iff q >= k
    wm = const.tile([P, P], BF16)
    nc.gpsimd.memset(wm, 1.0)
    nc.gpsimd.affine_select(
        out=wm,
        in_=wm,
        compare_op=mybir.AluOpType.is_ge,  # keep where f - p >= 0
        fill=0.0,
        base=0,
        channel_multiplier=-1,
        pattern=[[1, P]],
    )
    oc = const.tile([1, HD], BF16)  # ones row for rowsum-reciprocal broadcast
    nc.gpsimd.memset(oc, 1.0)

    big = ctx.enter_context(tc.tile_pool(name="big", bufs=1))
    xT = big.tile([P, 8, T], BF16)      # [d%128, d//128, t]
    QT = big.tile([P, 4, T], BF16)      # head h -> rows (h%2)*64.., subtile h//2
    KT = big.tile([P, 4, T], BF16)
    V_aug = big.tile([P, 16, H_LOC, HD + 1], BF16)  # [j%128, jb, h, dd|ones]
    AT = big.tile([P, 4, T], BF16)      # attention output, laid out like QT
    Qp0 = big.tile([P, T], BF16)        # padded Q scratch, even heads
    Qp1 = big.tile([P, T], BF16)        # padded Q scratch, odd heads
    nc.gpsimd.memset(V_aug[:, :, :, HD], 1.0)
    nc.gpsimd.memset(Qp0[64:128, :], 0.0)
    nc.gpsimd.memset(Qp1[0:64, :], 0.0)

    xa = x_ap.rearrange("(tb p) d -> tb p d", p=P)          # [16, 128, 1024]
    wqk = wqkv_ap[:, 0:2 * CLOC].rearrange("(o p) c -> p o c", p=P)
    wv = wqkv_ap[:, 2 * CLOC:3 * CLOC].rearrange("(o p) c -> p o c", p=P)
    wo = wout_ap.rearrange("(o p) n -> p o n", p=P)         # [128, 4, 1024]
    oa = out_ap.rearrange("(tb p) d -> tb p d", p=P)

    wpre = ctx.enter_context(tc.tile_pool(name="wpre", bufs=1))
    wv_sb = wpre.tile([P, 8, CLOC], BF16)
    wo_sb = wpre.tile([P, 4, D], BF16)

    # ---- phase B machinery (pair 0 is interleaved into phase A) ----
    ldw = ctx.enter_context(tc.tile_pool(name="ldw", bufs=2))
    psB = ctx.enter_context(tc.tile_pool(name="psB", bufs=1, space="PSUM"))

    def b_load_pair(pair):
        tiles = []
        for cb in (pair, pair + 4):
            wst = ldw.tile([P, 8, P], F32, tag="wst")
            nc.sync.dma_start(wst, wqk[:, :, cb * P:(cb + 1) * P])
            wcb = ldw.tile([P, 8, P], BF16, tag="wcb")
            nc.vector.tensor_copy(wcb, wst)
            tiles.append(wcb)
        return tiles

    def b_group(pair, tiles, gi):
        ci, it = divmod(gi, 4)
        cb = pair + 4 * ci
        dest = QT if cb < 4 else KT
        sub = cb % 4
        ps = psB.tile([P, 512], F32, tag="psb")
        for k in range(8):
            nc.tensor.matmul(
                ps,
                tiles[ci][:, k, :],
                xT[:, k, it * 512:(it + 1) * 512],
                start=(k == 0),
                stop=(k == 7),
            )
        nc.vector.tensor_copy(dest[:, sub, it * 512:(it + 1) * 512], ps)

    # ---- phase A: x -> xT (cast+transpose), V projection fused,
    # Q/K projection for head-pair 0 interleaved ----
    with tc.tile_pool(name="stage", bufs=1) as stage, \
         tc.tile_pool(name="lda", bufs=4) as lda, \
         tc.tile_pool(name="psA", bufs=4, space="PSUM") as psA, \
         tc.tile_pool(name="psV", bufs=2, space="PSUM") as psV:

        def load_x(tb):
            xin = lda.tile([P, D], F32, tag="xin")
            nc.sync.dma_start(xin, xa[tb])
            xc = lda.tile([P, D], BF16, tag="xc")
            nc.scalar.copy(xc, xin)  # cast on ScalarE; DVE is busier here
            return xc

        def vproj(tb):
            ps = psV.tile([P, CLOC], F32, tag="ps_v")
            for k in range(8):
                nc.tensor.matmul(
                    ps,
                    xT[:, k, tb * P:(tb + 1) * P],
                    wv_sb[:, k, :],
                    start=(k == 0),
                    stop=(k == 7),
                )
            nc.vector.tensor_copy(
                V_aug[:, tb, :, 0:HD],
                ps.rearrange("p (h d) -> p h d", h=H_LOC),
            )

        xc_cur = load_x(0)
        # V-proj weights in 4 chunks so the first vproj isn't gated on the
        # whole 2MB load; wo staged now, cast after the loop
        for j in range(4):
            wv_st = stage.tile([P, 2, CLOC], F32, tag=f"wv_st{j}")
            nc.sync.dma_start(wv_st, wv[:, 2 * j:2 * j + 2, :])
            nc.vector.tensor_copy(wv_sb[:, 2 * j:2 * j + 2, :], wv_st)
        wo_st = stage.tile([P, 4, D], F32, tag="wo_st")
        nc.sync.dma_start(wo_st, wo)

        pair0 = None
        for tb in range(T // P):
            xc_next = load_x(tb + 1) if tb + 1 < T // P else None
            if tb == 2:
                pair0 = b_load_pair(0)
            # one psum tile per transpose: a matmul with start=True clears
            # the whole destination bank, so slices of one bank can't be
            # written by separate transposes
            for db in range(8):
                pt = psA.tile([P, P], BF16, tag="pt")
                nc.tensor.transpose(pt, xc_cur[:, db * P:(db + 1) * P], ident)
                nc.vector.tensor_copy(xT[:, db, tb * P:(tb + 1) * P], pt)
            if tb > 0:
                vproj(tb - 1)  # one-deep pipeline behind the transposes
            if tb % 4 == 3:
                it = tb // 4
                b_group(0, pair0, it)      # cb=0 (Q sub 0)
                b_group(0, pair0, 4 + it)  # cb=4 (K sub 0)
            xc_cur = xc_next
        vproj(T // P - 1)
        nc.vector.tensor_copy(wo_sb, wo_st)

    # ---- phase C: causal attention ----
    # Scores matmuls contract over K=128 partitions (K<96 never warms the
    # PE HAM clock gate).  KT is packed (2 heads = 128 rows) as lhsT; the
    # moving Q operand is a per-parity scratch with the other head's 64
    # rows zeroed, so the packed-KT contraction picks out exactly one head.
    attp = ctx.enter_context(tc.tile_pool(name="att", bufs=3))
    smp = ctx.enter_context(tc.tile_pool(name="sm", bufs=2))
    ypool = ctx.enter_context(tc.tile_pool(name="ypool", bufs=3))
    psS = ctx.enter_context(tc.tile_pool(name="psS", bufs=2, space="PSUM"))
    psO = ctx.enter_context(tc.tile_pool(name="psO", bufs=2, space="PSUM"))

    def norm(pend):
        # softmax normalization for a finished (head, window) block
        po, row0, sub, i0 = pend
        rs = smp.tile([1, 512], F32, tag="rs")
        # copy to partition 0 first: reciprocal_approx_fast (custom DVE op)
        # mishandles a nonzero input partition offset
        nc.vector.tensor_copy(rs, po[HD:HD + 1, :])
        rr = smp.tile([1, 512], F32, tag="rr")
        nc.vector.reciprocal_approx_fast(rr, rs)
        rm = smp.tile([1, 512], BF16, tag="rm")
        nc.vector.tensor_copy(rm, rr)
        pb = psB.tile([HD, 512], F32, tag="pb")
        nc.tensor.matmul(pb, oc, rm, start=True, stop=True)
        rb = smp.tile([HD, 512], BF16, tag="rb")
        nc.vector.tensor_copy(rb, pb)
        nc.vector.tensor_tensor(
            AT[row0:row0 + 64, sub, i0:i0 + 512], po[0:HD, :], rb, mult
        )

    def d_piece(tb, nt, on_scalar=False):
        # output projection for one (query block, half) piece
        py = psB.tile([P, 512], F32, tag="psb")
        for k in range(4):
            nc.tensor.matmul(
                py,
                AT[:, k, tb * P:(tb + 1) * P],
                wo_sb[:, k, nt * 512:(nt + 1) * 512],
                start=(k == 0),
                stop=(k == 3),
            )
        ysb = ypool.tile([P, 512], F32, tag="ysb")
        if on_scalar:
            nc.scalar.copy(ysb, py)
        else:
            nc.vector.tensor_copy(ysb, py)
        nc.sync.dma_start(oa[tb, :, nt * 512:(nt + 1) * 512], ysb)

    dq = deque()

    def flush(pending):
        # emit deferred softmax normalization; once head 7's window `it` is
        # normalized, the out-projection for that window becomes runnable
        h, po, row0, sub, i0 = pending
        norm((po, row0, sub, i0))
        if h == 7:
            for tb in range(4 * (i0 // 512), 4 * (i0 // 512) + 4):
                for nt in range(2):
                    dq.append((tb, nt))

    pending = None
    for sub in range(4):
        ntiles = b_load_pair(sub + 1) if sub < 3 else None
        slot = 0
        for parity in (0, 1):
            row0 = parity * 64
            Qph = Qp0 if parity == 0 else Qp1
            nc.vector.tensor_copy(
                Qph[row0:row0 + 64, :], QT[row0:row0 + 64, sub, :]
            )
        for it in range(4):
            for parity in (0, 1):
                h = 2 * sub + parity
                row0 = parity * 64
                Qph = Qp0 if parity == 0 else Qp1
                if pending is not None:
                    flush(pending)
                    pending = None
                i0 = it * 512
                njb = 4 * (it + 1)
                po = psO.tile([P, 512], F32, tag="po")
                for jb2 in range(njb // 2):
                    ps = psS.tile([P, 1024], F32, tag="ps_s")
                    es = attp.tile([P, 1024], BF16, tag="es")
                    for u in (0, 1):
                        jb = 2 * jb2 + u
                        off = max(jb * P - i0, 0)
                        nc.tensor.matmul(
                            ps[:, u * 512 + off:(u + 1) * 512],
                            KT[:, sub, jb * P:(jb + 1) * P],
                            Qph[:, i0 + off:i0 + 512],
                            start=True,
                            stop=True,
                        )
                    # exp also covers stale cols left of the diagonal; they
                    # are never read downstream
                    nc.scalar.activation(es, ps, Exp, scale=0.125)
                    for u in (0, 1):
                        jb = 2 * jb2 + u
                        off = jb * P - i0
                        if off >= 0:  # 128x128 diagonal triangle
                            nc.vector.tensor_tensor(
                                es[:, u * 512 + off:u * 512 + off + P],
                                es[:, u * 512 + off:u * 512 + off + P],
                                wm,
                                mult,
                            )
                    for u in (0, 1):
                        jb = 2 * jb2 + u
                        off = max(jb * P - i0, 0)
                        nc.tensor.matmul(
                            po[0:HD + 1, off:512],
                            V_aug[:, jb, h, :],
                            es[:, u * 512 + off:(u + 1) * 512],
                            start=(jb == 0),
                            stop=(jb == njb - 1),
                        )
                    if dq:
                        d_piece(*dq.popleft())
                pending = (h, po, row0, sub, i0)
                if ntiles is not None and slot < 8:
                    b_group(sub + 1, ntiles, slot)
                    slot += 1
    flush(pending)
    k = 0
    while dq:
        d_piece(*dq.popleft(), on_scalar=(k % 2 == 1))
        k += 1

    ctx.close()


_CACHE = {}


def _get_nc(mode=None):
    if "nc" in _CACHE:
        return _CACHE["nc"]
    nc = bacc.Bacc(
        "TRN2",
        target_bir_lowering=False,
        debug=False,
        enable_asserts=False,
        num_devices=N_CORES,
    )
    x_d = nc.dram_tensor("x", [T, D], F32, kind="ExternalInput")
    wqkv_d = nc.dram_tensor("w_qkv", [D, 3 * CLOC], F32, kind="ExternalInput")
    wout_d = nc.dram_tensor("w_out", [CLOC, D], F32, kind="ExternalInput")
    out_d = nc.dram_tensor("out", [T, D], F32, kind="ExternalOutput")
    with tile.TileContext(nc) as tc:
        _build_kernel_body(
            nc, tc, x_d.ap(), wqkv_d.ap(), wout_d.ap(), out_d.ap()
        )
    nc.compile()
    _CACHE["nc"] = nc
    return nc


def _make_in_maps(x, w_qkv, w_out):
    x = np.ascontiguousarray(np.asarray(x, dtype=np.float32))
    w_qkv = np.ascontiguousarray(np.asarray(w_qkv, dtype=np.float32))
    w_out = np.ascontiguousarray(np.asarray(w_out, dtype=np.float32))
    in_maps = []
    for c in range(N_CORES):
        b, g = divmod(c, 2)
        c0 = g * CLOC
        wloc = np.concatenate(
            [
                w_qkv[:, c0:c0 + CLOC],
                w_qkv[:, D + c0:D + c0 + CLOC],
                w_qkv[:, 2 * D + c0:2 * D + c0 + CLOC],
            ],
            axis=1,
        )
        in_maps.append({
            "x": np.ascontiguousarray(x[b]),
            "w_qkv": np.ascontiguousarray(wloc),
            "w_out": np.ascontiguousarray(w_out[c0:c0 + CLOC]),
        })
    return in_maps


def run(x, w_qkv, w_out, trace=False, mode=None):
    nc = _get_nc(mode)
    in_maps = _make_in_maps(x, w_qkv, w_out)
    res = bass_utils.run_bass_kernel_spmd(
        nc, in_maps, core_ids=list(range(N_CORES)), trace=trace
    )
    y = np.empty((B, T, D), dtype=np.float32)
    for b in range(B):
        y[b] = res.results[2 * b]["out"] + res.results[2 * b + 1]["out"]
    return y, res


def kernel(x, w_qkv, w_out):
    y, _ = run(x, w_qkv, w_out, trace=False)
    return y


# revision 14
# speedup vs baseline: 1.6994x; 1.0613x over previous
"""Causal self-attention (B=4, T=2048, D=1024, H=16) on 8 TRN2 NeuronCores.

Sharding: core c handles batch b = c//2 and head-group g = c%2 (8 heads each).
Each core computes, for its (b, g):
    qkv_loc = x[b] @ w_qkv[:, cols(g)]          (q|k|v local, 512 cols each)
    att     = causal_attention(q, k, v)          (8 heads, hd=64)
    y_part  = att @ w_out[rows(g), :]            ([2048, 1024] partial)
Host sums the two partial outputs per batch.

All matmuls run in bf16 with fp32 PSUM accumulation. Softmax uses exp on
ScalarE with deferred normalization: rowsums come free from a ones-column
appended to V, the reciprocal is a single-pass Newton-Raphson approximation
read straight out of PSUM, and the result is broadcast across partitions
with a K=1 outer-product matmul.

Phase structure (single emission stream; engines overlap via Tile deps):
  A  x -> xT (cast on ScalarE + PE transpose), V projection fused in
  B0 Q/K projection for head-pair 0
  C  per head-pair `sub`: attention; Q/K projection for pair sub+1 is
     interleaved one matmul-group per (head, window) block so the PE stays
     fed while ScalarE runs exp; pair 3 interleaves the output projection
     instead.  Softmax normalization for block i is emitted during block
     i+1 (one-deep software pipeline) so its DVE chain never stalls the PE.
Causal masking: key-blocks fully above the diagonal are skipped; the
scores matmul / attention-V matmul are narrowed to the live band and only
the 128x128 diagonal triangle gets a mask multiply.
"""

import numpy as np
from collections import deque
from contextlib import ExitStack

import concourse.bass as bass
import concourse.mybir as mybir
from concourse import bacc, tile
from concourse import bass_utils
from concourse.masks import make_identity

# Problem constants (hardcoded per contest contract)
B = 4
T = 2048
D = 1024
H = 16
HD = 64
H_LOC = 8               # heads per core
CLOC = H_LOC * HD       # 512 local head dims
P = 128
N_CORES = 8

F32 = mybir.dt.float32
BF16 = mybir.dt.bfloat16
MM_MODE = "bf16"


def _build_kernel_body(nc, tc, x_ap, wqkv_ap, wout_ap, out_ap):
    Exp = mybir.ActivationFunctionType.Exp
    mult = mybir.AluOpType.mult

    ctx = ExitStack()

    # ---------------- constants ----------------
    const = ctx.enter_context(tc.tile_pool(name="const", bufs=1))
    ident = const.tile([P, P], BF16)
    make_identity(nc, ident)
    # causal keep-mask for a 128x128 diagonal block: wm[k, q] = 1.0 iff q >= k
    wm = const.tile([P, P], BF16)
    nc.gpsimd.memset(wm, 1.0)
    nc.gpsimd.affine_select(
        out=wm,
        in_=wm,
        compare_op=mybir.AluOpType.is_ge,  # keep where f - p >= 0
        fill=0.0,
        base=0,
        channel_multiplier=-1,
        pattern=[[1, P]],
    )
    oc = const.tile([1, HD], BF16)  # ones row for rowsum-reciprocal broadcast
    nc.gpsimd.memset(oc, 1.0)

    big = ctx.enter_context(tc.tile_pool(name="big", bufs=1))
    xT = big.tile([P, 8, T], BF16)      # [d%128, d//128, t]
    QT = big.tile([P, 4, T], BF16)      # head h -> rows (h%2)*64.., subtile h//2
    KT = big.tile([P, 4, T], BF16)
    V_aug = big.tile([P, 16, H_LOC, HD + 1], BF16)  # [j%128, jb, h, dd|ones]
    AT = big.tile([P, 4, T], BF16)      # attention output, laid out like QT
    Qp0 = big.tile([P, T], BF16)        # padded Q scratch, even heads
    Qp1 = big.tile([P, T], BF16)        # padded Q scratch, odd heads
    nc.gpsimd.memset(V_aug[:, :, :, HD], 1.0)
    nc.gpsimd.memset(Qp0[64:128, :], 0.0)
    nc.gpsimd.memset(Qp1[0:64, :], 0.0)

    xa = x_ap.rearrange("(tb p) d -> tb p d", p=P)          # [16, 128, 1024]
    wqk = wqkv_ap[:, 0:2 * CLOC].rearrange("(o p) c -> p o c", p=P)
    wv = wqkv_ap[:, 2 * CLOC:3 * CLOC].rearrange("(o p) c -> p o c", p=P)
    wo = wout_ap.rearrange("(o p) n -> p o n", p=P)         # [128, 4, 1024]
    oa = out_ap.rearrange("(tb p) d -> tb p d", p=P)

    wpre = ctx.enter_context(tc.tile_pool(name="wpre", bufs=1))
    wv_sb = wpre.tile([P, 8, CLOC], BF16)
    wo_sb = wpre.tile([P, 4, D], BF16)
    wo_st = wpre.tile([P, 4, D], F32)

    # ---- phase B machinery (pair 0 is interleaved into phase A) ----
    ldw = ctx.enter_context(tc.tile_pool(name="ldw", bufs=2))
    psB = ctx.enter_context(tc.tile_pool(name="psB", bufs=1, space="PSUM"))

    def b_load_pair(pair):
        tiles = []
        for cb in (pair, pair + 4):
            wst = ldw.tile([P, 8, P], F32, tag="wst")
            nc.sync.dma_start(wst, wqk[:, :, cb * P:(cb + 1) * P])
            wcb = ldw.tile([P, 8, P], BF16, tag="wcb")
            nc.vector.tensor_copy(wcb, wst)
            tiles.append(wcb)
        return tiles

    def b_group(pair, tiles, gi):
        ci, it = divmod(gi, 4)
        cb = pair + 4 * ci
        dest = QT if cb < 4 else KT
        sub = cb % 4
        ps = psB.tile([P, 512], F32, tag="psb")
        for k in range(8):
            nc.tensor.matmul(
                ps,
                tiles[ci][:, k, :],
                xT[:, k, it * 512:(it + 1) * 512],
                start=(k == 0),
                stop=(k == 7),
            )
        nc.vector.tensor_copy(dest[:, sub, it * 512:(it + 1) * 512], ps)

    # ---- phase A: x -> xT (cast+transpose), V projection fused,
    # Q/K projection for head-pair 0 interleaved ----
    with tc.tile_pool(name="stage", bufs=1) as stage, \
         tc.tile_pool(name="lda", bufs=4) as lda, \
         tc.tile_pool(name="psA", bufs=4, space="PSUM") as psA, \
         tc.tile_pool(name="psV", bufs=2, space="PSUM") as psV:

        def load_x(tb):
            xin = lda.tile([P, D], F32, tag="xin")
            nc.sync.dma_start(xin, xa[tb])
            xc = lda.tile([P, D], BF16, tag="xc")
            nc.scalar.copy(xc, xin)  # cast on ScalarE; DVE is busier here
            return xc

        def vproj(tb):
            ps = psV.tile([P, CLOC], F32, tag="ps_v")
            for k in range(8):
                nc.tensor.matmul(
                    ps,
                    xT[:, k, tb * P:(tb + 1) * P],
                    wv_sb[:, k, :],
                    start=(k == 0),
                    stop=(k == 7),
                )
            nc.vector.tensor_copy(
                V_aug[:, tb, :, 0:HD],
                ps.rearrange("p (h d) -> p h d", h=H_LOC),
            )

        def load_wv(j):
            wv_st = stage.tile([P, 2, CLOC], F32, tag=f"wv_st{j}")
            nc.sync.dma_start(wv_st, wv[:, 2 * j:2 * j + 2, :])
            nc.vector.tensor_copy(wv_sb[:, 2 * j:2 * j + 2, :], wv_st)

        # V-proj weights in 4 chunks so the first vproj isn't gated on the
        # whole 2MB load and the x stream keeps DMA priority
        xc_cur = load_x(0)
        load_wv(0)
        load_wv(1)

        pair0 = None
        for tb in range(T // P):
            xc_next = load_x(tb + 1) if tb + 1 < T // P else None
            if tb == 0:
                load_wv(2)
                load_wv(3)
            if tb == 2:
                pair0 = b_load_pair(0)
            # one psum tile per transpose: a matmul with start=True clears
            # the whole destination bank, so slices of one bank can't be
            # written by separate transposes
            for db in range(8):
                pt = psA.tile([P, P], BF16, tag="pt")
                nc.tensor.transpose(pt, xc_cur[:, db * P:(db + 1) * P], ident)
                nc.vector.tensor_copy(xT[:, db, tb * P:(tb + 1) * P], pt)
            if tb > 0:
                vproj(tb - 1)  # one-deep pipeline behind the transposes
            if tb % 4 == 3:
                it = tb // 4
                b_group(0, pair0, it)      # cb=0 (Q sub 0)
                b_group(0, pair0, 4 + it)  # cb=4 (K sub 0)
            xc_cur = xc_next
        vproj(T // P - 1)

    # ---- phase C: causal attention ----
    # Scores matmuls contract over K=128 partitions (K<96 never warms the
    # PE HAM clock gate).  KT is packed (2 heads = 128 rows) as lhsT; the
    # moving Q operand is a per-parity scratch with the other head's 64
    # rows zeroed, so the packed-KT contraction picks out exactly one head.
    attp = ctx.enter_context(tc.tile_pool(name="att", bufs=3))
    smp = ctx.enter_context(tc.tile_pool(name="sm", bufs=2))
    ypool = ctx.enter_context(tc.tile_pool(name="ypool", bufs=3))
    psS = ctx.enter_context(tc.tile_pool(name="psS", bufs=2, space="PSUM"))
    psO = ctx.enter_context(tc.tile_pool(name="psO", bufs=2, space="PSUM"))

    def norm(pend):
        # softmax normalization for a finished (head, window) block
        po, row0, sub, i0 = pend
        rs = smp.tile([1, 512], F32, tag="rs")
        # copy to partition 0 first: reciprocal_approx_fast (custom DVE op)
        # mishandles a nonzero input partition offset
        nc.vector.tensor_copy(rs, po[HD:HD + 1, :])
        rr = smp.tile([1, 512], F32, tag="rr")
        nc.vector.reciprocal_approx_fast(rr, rs)
        rm = smp.tile([1, 512], BF16, tag="rm")
        nc.vector.tensor_copy(rm, rr)
        pb = psB.tile([HD, 512], F32, tag="pb")
        nc.tensor.matmul(pb, oc, rm, start=True, stop=True)
        rb = smp.tile([HD, 512], BF16, tag="rb")
        nc.vector.tensor_copy(rb, pb)
        nc.vector.tensor_tensor(
            AT[row0:row0 + 64, sub, i0:i0 + 512], po[0:HD, :], rb, mult
        )

    def d_piece(tb, nt, on_scalar=False, alt_bank=False):
        # output projection for one (query block, half) piece
        py = psB.tile([P, 512], F32, tag="pb" if alt_bank else "psb")
        for k in range(4):
            nc.tensor.matmul(
                py,
                AT[:, k, tb * P:(tb + 1) * P],
                wo_sb[:, k, nt * 512:(nt + 1) * 512],
                start=(k == 0),
                stop=(k == 3),
            )
        ysb = ypool.tile([P, 512], F32, tag="ysb")
        if on_scalar:
            nc.scalar.copy(ysb, py)
        else:
            nc.vector.tensor_copy(ysb, py)
        nc.sync.dma_start(oa[tb, :, nt * 512:(nt + 1) * 512], ysb)

    dq = deque()

    def flush(pending):
        # emit deferred softmax normalization; once head 7's window `it` is
        # normalized, the out-projection for that window becomes runnable
        h, po, row0, sub, i0 = pending
        norm((po, row0, sub, i0))
        if h == 7:
            for tb in range(4 * (i0 // 512), 4 * (i0 // 512) + 4):
                for nt in range(2):
                    dq.append((tb, nt))

    pending = None
    for sub in range(4):
        if sub == 1:
            # out-proj weights loaded now: late enough not to contend with
            # the phase-A x stream, early enough for the first d_piece
            nc.sync.dma_start(wo_st, wo)
            nc.vector.tensor_copy(wo_sb, wo_st)
        ntiles = b_load_pair(sub + 1) if sub < 3 else None
        slot = 0
        for parity in (0, 1):
            row0 = parity * 64
            Qph = Qp0 if parity == 0 else Qp1
            nc.vector.tensor_copy(
                Qph[row0:row0 + 64, :], QT[row0:row0 + 64, sub, :]
            )
        for it in range(4):
            for parity in (0, 1):
                h = 2 * sub + parity
                row0 = parity * 64
                Qph = Qp0 if parity == 0 else Qp1
                if pending is not None:
                    flush(pending)
                    pending = None
                i0 = it * 512
                njb = 4 * (it + 1)
                # cap pops in head 6's last block so head 7's (ACT-bound)
                # block keeps PE filler work
                pop_budget = 4 if (sub == 3 and it == 3 and parity == 0) else 99
                po = psO.tile([P, 512], F32, tag="po")
                for jb2 in range(njb // 2):
                    ps = psS.tile([P, 1024], F32, tag="ps_s")
                    es = attp.tile([P, 1024], BF16, tag="es")
                    for u in (0, 1):
                        jb = 2 * jb2 + u
                        off = max(jb * P - i0, 0)
                        nc.tensor.matmul(
                            ps[:, u * 512 + off:(u + 1) * 512],
                            KT[:, sub, jb * P:(jb + 1) * P],
                            Qph[:, i0 + off:i0 + 512],
                            start=True,
                            stop=True,
                        )
                    # exp also covers stale cols left of the diagonal; they
                    # are never read downstream
                    nc.scalar.activation(es, ps, Exp, scale=0.125)
                    for u in (0, 1):
                        jb = 2 * jb2 + u
                        off = jb * P - i0
                        if off >= 0:  # 128x128 diagonal triangle
                            nc.vector.tensor_tensor(
                                es[:, u * 512 + off:u * 512 + off + P],
                                es[:, u * 512 + off:u * 512 + off + P],
                                wm,
                                mult,
                            )
                    for u in (0, 1):
                        jb = 2 * jb2 + u
                        off = max(jb * P - i0, 0)
                        nc.tensor.matmul(
                            po[0:HD + 1, off:512],
                            V_aug[:, jb, h, :],
                            es[:, u * 512 + off:(u + 1) * 512],
                            start=(jb == 0),
                            stop=(jb == njb - 1),
                        )
                    if dq and pop_budget > 0:
                        d_piece(*dq.popleft())
                        pop_budget -= 1
                pending = (h, po, row0, sub, i0)
                if ntiles is not None and slot < 8:
                    b_group(sub + 1, ntiles, slot)
                    slot += 1
    flush(pending)
    k = 0
    while dq:
        d_piece(*dq.popleft(), on_scalar=(k % 2 == 1), alt_bank=(k % 2 == 1))
        k += 1

    ctx.close()


_CACHE = {}


def _get_nc(mode=None):
    if "nc" in _CACHE:
        return _CACHE["nc"]
    nc = bacc.Bacc(
        "TRN2",
        target_bir_lowering=False,
        debug=False,
        enable_asserts=False,
        num_devices=N_CORES,
    )
    x_d = nc.dram_tensor("x", [T, D], F32, kind="ExternalInput")
    wqkv_d = nc.dram_tensor("w_qkv", [D, 3 * CLOC], F32, kind="ExternalInput")
    wout_d = nc.dram_tensor("w_out", [CLOC, D], F32, kind="ExternalInput")
    out_d = nc.dram_tensor("out", [T, D], F32, kind="ExternalOutput")
    with tile.TileContext(nc) as tc:
        _build_kernel_body(
            nc, tc, x_d.ap(), wqkv_d.ap(), wout_d.ap(), out_d.ap()
        )
    nc.compile()
    _CACHE["nc"] = nc
    return nc


def _make_in_maps(x, w_qkv, w_out):
    x = np.ascontiguousarray(np.asarray(x, dtype=np.float32))
    w_qkv = np.ascontiguousarray(np.asarray(w_qkv, dtype=np.float32))
    w_out = np.ascontiguousarray(np.asarray(w_out, dtype=np.float32))
    in_maps = []
    for c in range(N_CORES):
        b, g = divmod(c, 2)
        c0 = g * CLOC
        wloc = np.concatenate(
            [
                w_qkv[:, c0:c0 + CLOC],
                w_qkv[:, D + c0:D + c0 + CLOC],
                w_qkv[:, 2 * D + c0:2 * D + c0 + CLOC],
            ],
            axis=1,
        )
        in_maps.append({
            "x": np.ascontiguousarray(x[b]),
            "w_qkv": np.ascontiguousarray(wloc),
            "w_out": np.ascontiguousarray(w_out[c0:c0 + CLOC]),
        })
    return in_maps


def run(x, w_qkv, w_out, trace=False, mode=None):
    nc = _get_nc(mode)
    in_maps = _make_in_maps(x, w_qkv, w_out)
    res = bass_utils.run_bass_kernel_spmd(
        nc, in_maps, core_ids=list(range(N_CORES)), trace=trace
    )
    y = np.empty((B, T, D), dtype=np.float32)
    for b in range(B):
        y[b] = res.results[2 * b]["out"] + res.results[2 * b + 1]["out"]
    return y, res


def kernel(x, w_qkv, w_out):
    y, _ = run(x, w_qkv, w_out, trace=False)
    return y
